# revision 43
# baseline (speedup 1.0000x reference)
"""Trainium2 Bass kernel for nn_MultiHeadAttention (N=2048, D=1024, H=16, causal).

Sharding: 16 heads split across 8 NeuronCores (2 heads/core, tensor-parallel
per the sharding hint).  Each core projects Q^T/K^T (its 128 head-dims x full
sequence) and V for its heads, computes causal attention in scores-transposed
layout ([nk, nq] blocks, softmax along the nk partition axis with no
max-subtraction; the denominator falls out of a ones-column appended to V),
applies its 128-row slice of Wo, and writes a full [2048, 1024] partial
output.  The host sums the 8 partials and adds bo.

All streamed tensors (q/k/v, weights, partial outputs) are fp16 on the wire
and in the PE: this halves HBM traffic vs fp32 and runs the PE at full rate
at any tile width.  Attention matmuls are causally trimmed at 128-column
granularity (diagonal-block matmuls/exps only cover columns >= the block
offset), and score tiles are paired two-to-a-PSUM-allocation so most exp
activations run 1024 wide.  Input DMAs are 12 large transfers (one per
tensor x 512-column tile); outputs are 16 row-block transfers.
"""
import os
import sys

for _p in ("/opt/trn_rl_repo", "/root/.axon_site/_ro/trn_rl_repo"):
    if os.path.isdir(_p) and _p not in sys.path:
        sys.path.append(_p)

import numpy as np

import concourse.bass as bass
import concourse.mybir as mybir
from concourse import bacc
from concourse.bass_utils import run_bass_kernel_spmd
from concourse.tile import TileContext
from contextlib import ExitStack

N = 2048
D = 1024
NCORES = 8
DL = 128

F32 = mybir.dt.float32
F16 = mybir.dt.float16


def build_nc(opts=None):
    nc = bacc.Bacc("TRN2", target_bir_lowering=False, debug=False,
                   num_devices=NCORES)

    qT = nc.dram_tensor("qT", [D, N], F16, kind="ExternalInput")
    kT = nc.dram_tensor("kT", [D, N], F16, kind="ExternalInput")
    vT = nc.dram_tensor("vT", [D, N], F16, kind="ExternalInput")
    wqkv_d = nc.dram_tensor("wqkv", [D, 3 * DL], F16, kind="ExternalInput")
    wo_d = nc.dram_tensor("wo", [DL, D], F16, kind="ExternalInput")
    bqk = nc.dram_tensor("bqk", [DL, 3], F32, kind="ExternalInput")
    out = nc.dram_tensor("out", [N, D], F16, kind="ExternalOutput")

    AF = mybir.ActivationFunctionType
    from concourse.masks import make_identity

    with TileContext(nc) as tc, ExitStack() as ctx:
        const = ctx.enter_context(tc.tile_pool(name="const", bufs=1))
        big = ctx.enter_context(tc.tile_pool(name="big", bufs=1))
        stream = ctx.enter_context(tc.tile_pool(name="stream", bufs=1))
        vstage = ctx.enter_context(tc.tile_pool(name="vstage", bufs=2))
        probs_pool = ctx.enter_context(tc.tile_pool(name="probs", bufs=8))
        rc_pool = ctx.enter_context(tc.tile_pool(name="rc", bufs=2))
        outp = ctx.enter_context(tc.tile_pool(name="outp", bufs=16))

        # ---- PE pre-roll: cheap matmuls ahead of the first data-dependent
        # ones (the scheduler's issue pipeline works ahead on these) ----
        dummy = const.tile([16, 64], F16)
        nc.vector.memset(dummy[:], 0.0)
        with tc.tile_pool(name="warm_ps", bufs=1, space="PSUM") as warm_ps:
            wps0 = warm_ps.tile([16, 64], F32)
            for _ in range(85):
                nc.tensor.matmul(wps0[:], dummy[:, 0:16], dummy[:],
                                 start=True, stop=True)

        # ---- all DMAs on the sync queue, in exact consumption order; input
        # tiles are split in half (contraction chunks 0-3 / 4-7) so each
        # projection chain can start on the first half ----
        wqkv = const.tile([128, 8, 3 * DL], F16)
        qs = [[stream.tile([128, 4, 512], F16, name=f"q{t}{i}")
               for i in range(2)] for t in range(4)]
        ks = [[stream.tile([128, 4, 512], F16, name=f"k{t}{i}")
               for i in range(2)] for t in range(4)]
        vs = [[stream.tile([128, 4, 512], F16, name=f"v{t}{i}")
               for i in range(2)] for t in range(4)]
        qTr = qT.rearrange("(j p) n -> p j n", p=128)
        kTr = kT.rearrange("(j p) n -> p j n", p=128)
        vTr = vT.rearrange("(j p) n -> p j n", p=128)
        bias_cols = const.tile([128, 3], F32)
        wo = const.tile([128, D], F16)

        nc.sync.dma_start(wqkv[:],
                          wqkv_d.rearrange("(j p) c -> p j c", p=128))
        nc.sync.dma_start(bias_cols[:], bqk[:])
        # t=0 q/k halves interleaved so both projection chains start early
        nc.sync.dma_start(qs[0][0][:], qTr[:, 0:4, 0:512])
        nc.sync.dma_start(ks[0][0][:], kTr[:, 0:4, 0:512])
        nc.sync.dma_start(qs[0][1][:], qTr[:, 4:8, 0:512])
        nc.sync.dma_start(ks[0][1][:], kTr[:, 4:8, 0:512])
        nc.sync.dma_start(vs[0][0][:], vTr[:, 0:4, 0:512])
        nc.sync.dma_start(vs[0][1][:], vTr[:, 4:8, 0:512])
        nc.sync.dma_start(wo[:], wo_d[:])
        for t in range(1, 4):
            for xs, xr in ((qs, qTr), (ks, kTr), (vs, vTr)):
                for h in range(2):
                    nc.sync.dma_start(
                        xs[t][h][:],
                        xr[:, 4 * h:4 * (h + 1), 512 * t:512 * (t + 1)])

        ident = const.tile([128, 128], F32)
        make_identity(nc, ident[:])
        ones64 = const.tile([1, 64], F16)
        nc.vector.memset(ones64[:], 1.0)

        # persistent SBUF state
        QTs = [big.tile([128, 512], F16, name=f"QT{t}") for t in range(4)]
        KTs = [big.tile([128, 512], F16, name=f"KT{t}") for t in range(4)]
        Vaug0 = big.tile([128, 16, 65], F16)
        Vaug1 = big.tile([128, 16, 65], F16)
        nc.vector.memset(Vaug0[:, :, 64:65], 1.0)
        nc.vector.memset(Vaug1[:, :, 64:65], 1.0)
        attnT = big.tile([128, N], F16)

        def wo_block(m, pool):
            ob = outp.tile([128, 1024], F16, name="ob")
            for u in range(2):
                wps = pool.tile([128, 512], F32, name="mm")
                nc.tensor.matmul(wps[:], attnT[:, 128 * m:128 * (m + 1)],
                                 wo[:, 512 * u:512 * (u + 1)],
                                 start=True, stop=True)
                nc.vector.tensor_copy(ob[:, 512 * u:512 * (u + 1)],
                                      wps[:])
            nc.sync.dma_start(out[128 * m:128 * (m + 1), :], ob[:])

        with tc.tile_pool(name="sc_ps", bufs=2, space="PSUM") as sc_ps, \
             tc.tile_pool(name="pv_ps", bufs=2, space="PSUM") as pv_ps, \
             tc.tile_pool(name="mm_ps", bufs=2, space="PSUM") as mm_ps:

            def qk_proj(t, interleave=False):
                specs = ((0, qs[t], 0, QTs[t]), (DL, ks[t], 1, KTs[t]))
                if not interleave:
                    for c0, src, bcol, dst in specs:
                        ps = mm_ps.tile([128, 512], F32, name="mm")
                        for j in range(8):
                            nc.tensor.matmul(ps[:], wqkv[:, j, c0:c0 + DL],
                                             src[j // 4][:, j % 4, :],
                                             start=(j == 0), stop=(j == 7))
                        nc.vector.tensor_scalar_add(
                            dst[:], ps[:], bias_cols[:, bcol:bcol + 1])
                    return
                # consume input halves in arrival order: Qlo, Klo, Qhi, Khi
                pss = [mm_ps.tile([128, 512], F32, name="mm")
                       for _ in range(2)]
                for half in range(2):
                    for i, (c0, src, bcol, dst) in enumerate(specs):
                        for j in range(4 * half, 4 * half + 4):
                            nc.tensor.matmul(pss[i][:],
                                             wqkv[:, j, c0:c0 + DL],
                                             src[j // 4][:, j % 4, :],
                                             start=(j == 0), stop=(j == 7))
                for i, (c0, src, bcol, dst) in enumerate(specs):
                    nc.vector.tensor_scalar_add(
                        dst[:], pss[i][:], bias_cols[:, bcol:bcol + 1])

            def v_proj(t):
                ps = mm_ps.tile([128, 512], F32, name="mm")
                for j in range(8):
                    nc.tensor.matmul(ps[:], wqkv[:, j, 2 * DL:3 * DL],
                                     vs[t][j // 4][:, j % 4, :],
                                     start=(j == 0), stop=(j == 7))
                vtt = vstage.tile([128, 512], F32, name="vtt")
                nc.vector.tensor_scalar_add(vtt[:], ps[:], bias_cols[:, 2:3])
                for bb in range(4):
                    b = 4 * t + bb
                    tp = mm_ps.tile([128, 512], F32, name="mm")
                    nc.tensor.transpose(tp[:, 0:128],
                                        vtt[:, 128 * bb:128 * (bb + 1)],
                                        ident[:])
                    nc.vector.tensor_copy(Vaug0[:, b, 0:64], tp[:, 0:64])
                    nc.vector.tensor_copy(Vaug1[:, b, 0:64], tp[:, 64:128])

            def score_mm(t, h, kb, sc, off):
                nc.tensor.matmul(
                    sc[:, kb % 2, off:512],
                    KTs[kb // 4][64 * h:64 * (h + 1),
                                 128 * (kb % 4):128 * (kb % 4 + 1)],
                    QTs[t][64 * h:64 * (h + 1), off:512],
                    start=True, stop=True, tile_position=(64 * h, 0))

            def drain_pv(t, h, pvh, pend, keep, max_kb=None):
                # kb-sorted: kb=0 must go first (it owns has_written for the
                # whole accumulation); max_kb guards blocks whose V isn't
                # projected yet
                nkb = 4 * t + 4
                pend.sort(key=lambda e: e[0])
                while len(pend) > keep and (max_kb is None
                                            or pend[0][0] < max_kb):
                    pkb, ppr, pgi, poff = pend.pop(0)
                    nc.tensor.matmul(pvh[:, poff:512],
                                     Vaug_[h][:, pkb, :],
                                     ppr[:, pgi, poff:512],
                                     start=(pkb == 0), stop=(pkb == nkb - 1))

            Vaug_ = (Vaug0, Vaug1)

            # one iteration = one 512-column query tile; emission is ordered
            # by data arrival: full score blocks (need only old V) before the
            # V projection, next Q/K projection before the Wo matmuls
            qk_proj(0, interleave=True)
            for t in range(4):
                nkb = 4 * t + 4
                pvhs = [pv_ps.tile([65, 512], F32, name="pvh")
                        for _ in range(2)]
                pends = [[], []]
                # previous tile's output projections interleave with this
                # tile's full-block scores (their evacs hide behind the PE)
                wo_pend = list(range(4 * (t - 1), 4 * t)) if t >= 1 else []
                # full blocks (kb < 4t), both heads
                for h in range(2):
                    for kb in range(4 * t):
                        if kb % 2 == 0:
                            sc = sc_ps.tile([128, 2, 512], F32, name="sc")
                            pr = probs_pool.tile([128, 2, 512], F16,
                                                 name="pr")
                        score_mm(t, h, kb, sc, 0)
                        if kb % 2 == 1:
                            nc.scalar.activation(pr[:, :, :], sc[:, :, :],
                                                 AF.Exp, scale=0.125)
                            pends[h].append((kb - 1, pr, 0, 0))
                            pends[h].append((kb, pr, 1, 0))
                            if wo_pend:
                                wo_block(wo_pend.pop(0), mm_ps)
                        drain_pv(t, h, pvhs[h], pends[h], 4, max_kb=4 * t)
                for m in wo_pend:
                    wo_block(m, mm_ps)

                def diag_scores(t):
                    for h in range(2):
                        for kb in range(4 * t, nkb):
                            if kb % 2 == 0:
                                sc = sc_ps.tile([128, 2, 512], F32,
                                                name="sc")
                                pr = probs_pool.tile([128, 2, 512], F16,
                                                     name="pr")
                            off = 128 * (kb - 4 * t)
                            score_mm(t, h, kb, sc, off)
                            nc.scalar.activation(pr[:, kb % 2, off:512],
                                                 sc[:, kb % 2, off:512],
                                                 AF.Exp, scale=0.125)
                            nc.gpsimd.affine_select(
                                out=pr[:, kb % 2, off:off + 128],
                                in_=pr[:, kb % 2, off:off + 128],
                                compare_op=mybir.AluOpType.is_ge, fill=0.0,
                                base=0, pattern=[[1, 128]],
                                channel_multiplier=-1)
                            pends[h].append((kb, pr, kb % 2, off))

                # t=0: v arrives last, so emit the diagonal scores (which
                # need only Q/K) before the V projection
                if t == 0:
                    diag_scores(t)
                    v_proj(t)
                else:
                    v_proj(t)
                    diag_scores(t)
                # next tile's Q/K projection fills the exp/mask latency
                if t < 3:
                    qk_proj(t + 1)
                # PV drains + softmax finalize, both heads
                for h in range(2):
                    drain_pv(t, h, pvhs[h], pends[h], 0)
                    # softmax normalization: denom row to SBUF, broadcast
                    # over 64 partitions via PE, reciprocal back to SBUF
                    # (so the multiply reads only one PSUM operand),
                    # multiply into attnT
                    dnr = rc_pool.tile([1, 512], F16, name="dnr")
                    nc.vector.tensor_copy(dnr[:], pvhs[h][64:65, :])
                    bcp = mm_ps.tile([128, 512], F32, name="mm")
                    nc.tensor.matmul(bcp[0:64, :], ones64[:], dnr[:],
                                     start=True, stop=True)
                    rcb = rc_pool.tile([64, 512], F16, name="rcb")
                    with nc.allow_low_precision(reason="softmax recip"):
                        nc.vector.reciprocal(rcb[:], bcp[0:64, :])
                    nc.vector.tensor_mul(
                        attnT[64 * h:64 * (h + 1), 512 * t:512 * (t + 1)],
                        pvhs[h][0:64, :], rcb[:])
        # ---- final output projection (row blocks 12..15) in its own PSUM
        # region: the attention pools are closed, so 8 banks decouple the
        # matmuls from their evacuations completely ----
        with tc.tile_pool(name="wo3_ps", bufs=8, space="PSUM") as wo3_ps:
            for m in range(12, 16):
                wo_block(m, wo3_ps)

    nc.compile()
    return nc


def make_in_maps(q, k, v, Wq, bq, Wk, bk, Wv, bv, Wo, bo):
    f16 = np.float16
    f32 = np.float32
    qTa = np.ascontiguousarray(q.T).astype(f16)
    kTa = np.ascontiguousarray(k.T).astype(f16)
    vTa = np.ascontiguousarray(v.T).astype(f16)
    WqT = Wq.T.astype(f16)
    WkT = Wk.T.astype(f16)
    WvT = Wv.T.astype(f16)
    WoT = Wo.T.astype(f16)
    in_maps = []
    for c in range(NCORES):
        d0 = DL * c
        in_maps.append({
            "qT": qTa, "kT": kTa, "vT": vTa,
            "wqkv": np.ascontiguousarray(
                np.concatenate([WqT[:, d0:d0 + DL], WkT[:, d0:d0 + DL],
                                WvT[:, d0:d0 + DL]], axis=1)),
            "wo": np.ascontiguousarray(WoT[d0:d0 + DL, :]),
            "bqk": np.ascontiguousarray(
                np.stack([bq[d0:d0 + DL], bk[d0:d0 + DL], bv[d0:d0 + DL]],
                         axis=1)).astype(f32),
        })
    return in_maps


_NC_CACHE = None


def _get_nc():
    global _NC_CACHE
    if _NC_CACHE is None:
        _NC_CACHE = build_nc()
    return _NC_CACHE


def kernel(q, k, v, Wq, bq, Wk, bk, Wv, bv, Wo, bo):
    """Full-input / full-output entry point (harness contract)."""
    q, k, v = np.asarray(q), np.asarray(k), np.asarray(v)
    Wq, bq, Wk, bk = np.asarray(Wq), np.asarray(bq), np.asarray(Wk), np.asarray(bk)
    Wv, bv, Wo, bo = np.asarray(Wv), np.asarray(bv), np.asarray(Wo), np.asarray(bo)
    nc = _get_nc()
    in_maps = make_in_maps(q, k, v, Wq, bq, Wk, bk, Wv, bv, Wo, bo)
    res = run_bass_kernel_spmd(nc, in_maps, list(range(NCORES)))
    acc = res.results[0]["out"].astype(np.float64)
    for c in range(1, NCORES):
        acc += res.results[c]["out"]
    acc += bo.astype(np.float64)
    return acc.astype(np.float32)


# revision 44
# speedup vs baseline: 1.0146x; 1.0146x over previous
"""Trainium2 Bass kernel for nn_MultiHeadAttention (N=2048, D=1024, H=16, causal).

Sharding: 16 heads split across 8 NeuronCores (2 heads/core, tensor-parallel
per the sharding hint).  Each core projects Q^T/K^T (its 128 head-dims x full
sequence) and V for its heads, computes causal attention in scores-transposed
layout ([nk, nq] blocks, softmax along the nk partition axis with no
max-subtraction; the denominator falls out of a ones-column appended to V),
applies its 128-row slice of Wo, and writes a full [2048, 1024] partial
output.  The host sums the 8 partials and adds bo.

All streamed tensors (q/k/v, weights, partial outputs) are fp16 on the wire
and in the PE: this halves HBM traffic vs fp32 and runs the PE at full rate
at any tile width.  Attention matmuls are causally trimmed at 128-column
granularity (diagonal-block matmuls/exps only cover columns >= the block
offset), and score tiles are paired two-to-a-PSUM-allocation so most exp
activations run 1024 wide.  Input DMAs are 12 large transfers (one per
tensor x 512-column tile); outputs are 16 row-block transfers.
"""
import os
import sys

for _p in ("/opt/trn_rl_repo", "/root/.axon_site/_ro/trn_rl_repo"):
    if os.path.isdir(_p) and _p not in sys.path:
        sys.path.append(_p)

import numpy as np

import concourse.bass as bass
import concourse.mybir as mybir
from concourse import bacc
from concourse.bass_utils import run_bass_kernel_spmd
from concourse.tile import TileContext
from contextlib import ExitStack

N = 2048
D = 1024
NCORES = 8
DL = 128

F32 = mybir.dt.float32
F16 = mybir.dt.float16


def build_nc(opts=None):
    nc = bacc.Bacc("TRN2", target_bir_lowering=False, debug=False,
                   num_devices=NCORES)

    qT = nc.dram_tensor("qT", [D, N], F16, kind="ExternalInput")
    kT = nc.dram_tensor("kT", [D, N], F16, kind="ExternalInput")
    vT = nc.dram_tensor("vT", [D, N], F16, kind="ExternalInput")
    wqkv_d = nc.dram_tensor("wqkv", [D, 3 * DL], F16, kind="ExternalInput")
    wo_d = nc.dram_tensor("wo", [DL, D], F16, kind="ExternalInput")
    bqk = nc.dram_tensor("bqk", [DL, 3], F32, kind="ExternalInput")
    out = nc.dram_tensor("out", [N, D], F16, kind="ExternalOutput")

    AF = mybir.ActivationFunctionType
    from concourse.masks import make_identity

    with TileContext(nc) as tc, ExitStack() as ctx:
        const = ctx.enter_context(tc.tile_pool(name="const", bufs=1))
        big = ctx.enter_context(tc.tile_pool(name="big", bufs=1))
        stream = ctx.enter_context(tc.tile_pool(name="stream", bufs=1))
        vstage = ctx.enter_context(tc.tile_pool(name="vstage", bufs=2))
        probs_pool = ctx.enter_context(tc.tile_pool(name="probs", bufs=8))
        rc_pool = ctx.enter_context(tc.tile_pool(name="rc", bufs=2))
        outp = ctx.enter_context(tc.tile_pool(name="outp", bufs=16))

        # ---- PE pre-roll: cheap matmuls ahead of the first data-dependent
        # ones (the scheduler's issue pipeline works ahead on these) ----
        dummy = const.tile([16, 64], F16)
        nc.vector.memset(dummy[:], 0.0)
        with tc.tile_pool(name="warm_ps", bufs=1, space="PSUM") as warm_ps:
            wps0 = warm_ps.tile([16, 64], F32)
            for _ in range(85):
                nc.tensor.matmul(wps0[:], dummy[:, 0:16], dummy[:],
                                 start=True, stop=True)

        # ---- all DMAs on the sync queue, in exact consumption order; input
        # tiles are split in half (contraction chunks 0-3 / 4-7) so each
        # projection chain can start on the first half ----
        wqkv = const.tile([128, 8, 3 * DL], F16)
        qs = [[stream.tile([128, 4, 512], F16, name=f"q{t}{i}")
               for i in range(2)] for t in range(4)]
        ks = [[stream.tile([128, 4, 512], F16, name=f"k{t}{i}")
               for i in range(2)] for t in range(4)]
        vs = [[stream.tile([128, 4, 512], F16, name=f"v{t}{i}")
               for i in range(2)] for t in range(4)]
        qTr = qT.rearrange("(j p) n -> p j n", p=128)
        kTr = kT.rearrange("(j p) n -> p j n", p=128)
        vTr = vT.rearrange("(j p) n -> p j n", p=128)
        bias_cols = const.tile([128, 3], F32)
        wo = const.tile([128, D], F16)

        nc.sync.dma_start(wqkv[:],
                          wqkv_d.rearrange("(j p) c -> p j c", p=128))
        nc.sync.dma_start(bias_cols[:], bqk[:])
        # t=0 q/k halves interleaved so both projection chains start early
        nc.sync.dma_start(qs[0][0][:], qTr[:, 0:4, 0:512])
        nc.sync.dma_start(ks[0][0][:], kTr[:, 0:4, 0:512])
        nc.sync.dma_start(qs[0][1][:], qTr[:, 4:8, 0:512])
        nc.sync.dma_start(ks[0][1][:], kTr[:, 4:8, 0:512])
        nc.sync.dma_start(vs[0][0][:], vTr[:, 0:4, 0:512])
        nc.sync.dma_start(vs[0][1][:], vTr[:, 4:8, 0:512])
        nc.sync.dma_start(wo[:], wo_d[:])
        for t in range(1, 4):
            for xs, xr in ((qs, qTr), (ks, kTr), (vs, vTr)):
                for h in range(2):
                    nc.sync.dma_start(
                        xs[t][h][:],
                        xr[:, 4 * h:4 * (h + 1), 512 * t:512 * (t + 1)])

        ident = const.tile([128, 128], F32)
        make_identity(nc, ident[:])
        ones64 = const.tile([1, 64], F16)
        nc.vector.memset(ones64[:], 1.0)

        # persistent SBUF state
        QTs = [big.tile([128, 512], F16, name=f"QT{t}") for t in range(4)]
        KTs = [big.tile([128, 512], F16, name=f"KT{t}") for t in range(4)]
        Vaug0 = big.tile([128, 16, 65], F16)
        Vaug1 = big.tile([128, 16, 65], F16)
        nc.vector.memset(Vaug0[:, :, 64:65], 1.0)
        nc.vector.memset(Vaug1[:, :, 64:65], 1.0)
        attnT = big.tile([128, N], F16)

        def wo_block(m, pool):
            ob = outp.tile([128, 1024], F16, name="ob")
            for u in range(2):
                wps = pool.tile([128, 512], F32, name="mm")
                nc.tensor.matmul(wps[:], attnT[:, 128 * m:128 * (m + 1)],
                                 wo[:, 512 * u:512 * (u + 1)],
                                 start=True, stop=True)
                nc.vector.tensor_copy(ob[:, 512 * u:512 * (u + 1)],
                                      wps[:])
            nc.sync.dma_start(out[128 * m:128 * (m + 1), :], ob[:])

        with tc.tile_pool(name="sc_ps", bufs=2, space="PSUM") as sc_ps, \
             tc.tile_pool(name="pv_ps", bufs=2, space="PSUM") as pv_ps, \
             tc.tile_pool(name="mm_ps", bufs=2, space="PSUM") as mm_ps:

            def qk_proj(t, interleave=False):
                specs = ((0, qs[t], 0, QTs[t]), (DL, ks[t], 1, KTs[t]))
                if not interleave:
                    for c0, src, bcol, dst in specs:
                        ps = mm_ps.tile([128, 512], F32, name="mm")
                        for j in range(8):
                            nc.tensor.matmul(ps[:], wqkv[:, j, c0:c0 + DL],
                                             src[j // 4][:, j % 4, :],
                                             start=(j == 0), stop=(j == 7))
                        nc.vector.tensor_scalar_add(
                            dst[:], ps[:], bias_cols[:, bcol:bcol + 1])
                    return
                # consume input halves in arrival order: Qlo, Klo, Qhi, Khi
                pss = [mm_ps.tile([128, 512], F32, name="mm")
                       for _ in range(2)]
                for half in range(2):
                    for i, (c0, src, bcol, dst) in enumerate(specs):
                        for j in range(4 * half, 4 * half + 4):
                            nc.tensor.matmul(pss[i][:],
                                             wqkv[:, j, c0:c0 + DL],
                                             src[j // 4][:, j % 4, :],
                                             start=(j == 0), stop=(j == 7))
                for i, (c0, src, bcol, dst) in enumerate(specs):
                    nc.vector.tensor_scalar_add(
                        dst[:], pss[i][:], bias_cols[:, bcol:bcol + 1])

            def v_proj(t):
                ps = mm_ps.tile([128, 512], F32, name="mm")
                for j in range(8):
                    nc.tensor.matmul(ps[:], wqkv[:, j, 2 * DL:3 * DL],
                                     vs[t][j // 4][:, j % 4, :],
                                     start=(j == 0), stop=(j == 7))
                vtt = vstage.tile([128, 512], F32, name="vtt")
                nc.vector.tensor_scalar_add(vtt[:], ps[:], bias_cols[:, 2:3])
                for bb in range(4):
                    b = 4 * t + bb
                    tp = mm_ps.tile([128, 512], F32, name="mm")
                    nc.tensor.transpose(tp[:, 0:128],
                                        vtt[:, 128 * bb:128 * (bb + 1)],
                                        ident[:])
                    nc.vector.tensor_copy(Vaug0[:, b, 0:64], tp[:, 0:64])
                    nc.vector.tensor_copy(Vaug1[:, b, 0:64], tp[:, 64:128])

            def score_mm(t, h, kb, sc, off):
                nc.tensor.matmul(
                    sc[:, kb % 2, off:512],
                    KTs[kb // 4][64 * h:64 * (h + 1),
                                 128 * (kb % 4):128 * (kb % 4 + 1)],
                    QTs[t][64 * h:64 * (h + 1), off:512],
                    start=True, stop=True, tile_position=(64 * h, 0))

            def drain_pv(t, h, pvh, pend, keep, max_kb=None):
                # kb-sorted: kb=0 must go first (it owns has_written for the
                # whole accumulation); max_kb guards blocks whose V isn't
                # projected yet
                nkb = 4 * t + 4
                pend.sort(key=lambda e: e[0])
                while len(pend) > keep and (max_kb is None
                                            or pend[0][0] < max_kb):
                    pkb, ppr, pgi, poff = pend.pop(0)
                    nc.tensor.matmul(pvh[:, poff:512],
                                     Vaug_[h][:, pkb, :],
                                     ppr[:, pgi, poff:512],
                                     start=(pkb == 0), stop=(pkb == nkb - 1))

            Vaug_ = (Vaug0, Vaug1)

            # one iteration = one 512-column query tile; emission is ordered
            # by data arrival: full score blocks (need only old V) before the
            # V projection, next Q/K projection before the Wo matmuls
            qk_proj(0, interleave=True)
            for t in range(4):
                nkb = 4 * t + 4
                pvhs = [pv_ps.tile([65, 512], F32, name="pvh")
                        for _ in range(2)]
                pends = [[], []]
                # previous tile's output projections interleave with this
                # tile's full-block scores (their evacs hide behind the PE)
                wo_pend = list(range(4 * (t - 1), 4 * t)) if t >= 1 else []
                # full blocks (kb < 4t), both heads
                for h in range(2):
                    for kb in range(4 * t):
                        if kb % 2 == 0:
                            sc = sc_ps.tile([128, 2, 512], F32, name="sc")
                            pr = probs_pool.tile([128, 2, 512], F16,
                                                 name="pr")
                        score_mm(t, h, kb, sc, 0)
                        if kb % 2 == 1:
                            nc.scalar.activation(pr[:, :, :], sc[:, :, :],
                                                 AF.Exp, scale=0.125)
                            pends[h].append((kb - 1, pr, 0, 0))
                            pends[h].append((kb, pr, 1, 0))
                            if wo_pend:
                                wo_block(wo_pend.pop(0), mm_ps)
                        drain_pv(t, h, pvhs[h], pends[h], 4, max_kb=4 * t)
                for m in wo_pend:
                    wo_block(m, mm_ps)

                def diag_scores(t):
                    for h in range(2):
                        for kb in range(4 * t, nkb):
                            if kb % 2 == 0:
                                sc = sc_ps.tile([128, 2, 512], F32,
                                                name="sc")
                                pr = probs_pool.tile([128, 2, 512], F16,
                                                     name="pr")
                            off = 128 * (kb - 4 * t)
                            score_mm(t, h, kb, sc, off)
                            nc.scalar.activation(pr[:, kb % 2, off:512],
                                                 sc[:, kb % 2, off:512],
                                                 AF.Exp, scale=0.125)
                            nc.gpsimd.affine_select(
                                out=pr[:, kb % 2, off:off + 128],
                                in_=pr[:, kb % 2, off:off + 128],
                                compare_op=mybir.AluOpType.is_ge, fill=0.0,
                                base=0, pattern=[[1, 128]],
                                channel_multiplier=-1)
                            pends[h].append((kb, pr, kb % 2, off))

                # t=0: v arrives last, so emit the diagonal scores (which
                # need only Q/K) before the V projection
                if t == 0:
                    diag_scores(t)
                    v_proj(t)
                else:
                    v_proj(t)
                    diag_scores(t)
                # next tile's Q/K projection fills the exp/mask latency
                if t < 3:
                    qk_proj(t + 1)
                # PV drains + softmax finalize, both heads
                for h in range(2):
                    drain_pv(t, h, pvhs[h], pends[h], 0)
                    # softmax normalization: denom row to SBUF, broadcast
                    # over 64 partitions via PE, reciprocal back to SBUF
                    # (so the multiply reads only one PSUM operand),
                    # multiply into attnT
                    dnr = rc_pool.tile([1, 512], F16, name="dnr")
                    nc.scalar.activation(dnr[:], pvhs[h][64:65, :], AF.Copy)
                    bcp = mm_ps.tile([128, 512], F32, name="mm")
                    nc.tensor.matmul(bcp[0:64, :], ones64[:], dnr[:],
                                     start=True, stop=True)
                    rcb = rc_pool.tile([64, 512], F16, name="rcb")
                    with nc.allow_low_precision(reason="softmax recip"):
                        nc.vector.reciprocal(rcb[:], bcp[0:64, :])
                    nc.vector.tensor_mul(
                        attnT[64 * h:64 * (h + 1), 512 * t:512 * (t + 1)],
                        pvhs[h][0:64, :], rcb[:])
        # ---- final output projection (row blocks 12..15) in its own PSUM
        # region: the attention pools are closed, so 8 banks decouple the
        # matmuls from their evacuations completely ----
        with tc.tile_pool(name="wo3_ps", bufs=8, space="PSUM") as wo3_ps:
            for m in range(12, 16):
                wo_block(m, wo3_ps)

    nc.compile()
    return nc


def make_in_maps(q, k, v, Wq, bq, Wk, bk, Wv, bv, Wo, bo):
    f16 = np.float16
    f32 = np.float32
    qTa = np.ascontiguousarray(q.T).astype(f16)
    kTa = np.ascontiguousarray(k.T).astype(f16)
    vTa = np.ascontiguousarray(v.T).astype(f16)
    WqT = Wq.T.astype(f16)
    WkT = Wk.T.astype(f16)
    WvT = Wv.T.astype(f16)
    WoT = Wo.T.astype(f16)
    in_maps = []
    for c in range(NCORES):
        d0 = DL * c
        in_maps.append({
            "qT": qTa, "kT": kTa, "vT": vTa,
            "wqkv": np.ascontiguousarray(
                np.concatenate([WqT[:, d0:d0 + DL], WkT[:, d0:d0 + DL],
                                WvT[:, d0:d0 + DL]], axis=1)),
            "wo": np.ascontiguousarray(WoT[d0:d0 + DL, :]),
            "bqk": np.ascontiguousarray(
                np.stack([bq[d0:d0 + DL], bk[d0:d0 + DL], bv[d0:d0 + DL]],
                         axis=1)).astype(f32),
        })
    return in_maps


_NC_CACHE = None


def _get_nc():
    global _NC_CACHE
    if _NC_CACHE is None:
        _NC_CACHE = build_nc()
    return _NC_CACHE


def kernel(q, k, v, Wq, bq, Wk, bk, Wv, bv, Wo, bo):
    """Full-input / full-output entry point (harness contract)."""
    q, k, v = np.asarray(q), np.asarray(k), np.asarray(v)
    Wq, bq, Wk, bk = np.asarray(Wq), np.asarray(bq), np.asarray(Wk), np.asarray(bk)
    Wv, bv, Wo, bo = np.asarray(Wv), np.asarray(bv), np.asarray(Wo), np.asarray(bo)
    nc = _get_nc()
    in_maps = make_in_maps(q, k, v, Wq, bq, Wk, bk, Wv, bv, Wo, bo)
    res = run_bass_kernel_spmd(nc, in_maps, list(range(NCORES)))
    acc = res.results[0]["out"].astype(np.float64)
    for c in range(1, NCORES):
        acc += res.results[c]["out"]
    acc += bo.astype(np.float64)
    return acc.astype(np.float32)


# revision 45
# speedup vs baseline: 1.0330x; 1.0182x over previous
"""Trainium2 Bass kernel for nn_MultiHeadAttention (N=2048, D=1024, H=16, causal).

Sharding: 16 heads split across 8 NeuronCores (2 heads/core, tensor-parallel
per the sharding hint).  Each core projects Q^T/K^T (its 128 head-dims x full
sequence) and V for its heads, computes causal attention in scores-transposed
layout ([nk, nq] blocks, softmax along the nk partition axis with no
max-subtraction; the denominator falls out of a ones-column appended to V),
applies its 128-row slice of Wo, and writes a full [2048, 1024] partial
output.  The host sums the 8 partials and adds bo.

All streamed tensors (q/k/v, weights, partial outputs) are fp16 on the wire
and in the PE: this halves HBM traffic vs fp32 and runs the PE at full rate
at any tile width.  Attention matmuls are causally trimmed at 128-column
granularity (diagonal-block matmuls/exps only cover columns >= the block
offset), and score tiles are paired two-to-a-PSUM-allocation so most exp
activations run 1024 wide.  Input DMAs are 12 large transfers (one per
tensor x 512-column tile); outputs are 16 row-block transfers.
"""
import os
import sys

for _p in ("/opt/trn_rl_repo", "/root/.axon_site/_ro/trn_rl_repo"):
    if os.path.isdir(_p) and _p not in sys.path:
        sys.path.append(_p)

import numpy as np

import concourse.bass as bass
import concourse.mybir as mybir
from concourse import bacc
from concourse.bass_utils import run_bass_kernel_spmd
from concourse.tile import TileContext
from contextlib import ExitStack

N = 2048
D = 1024
NCORES = 8
DL = 128

F32 = mybir.dt.float32
F16 = mybir.dt.float16


def build_nc(opts=None):
    nc = bacc.Bacc("TRN2", target_bir_lowering=False, debug=False,
                   num_devices=NCORES)

    qT = nc.dram_tensor("qT", [D, N], F16, kind="ExternalInput")
    kT = nc.dram_tensor("kT", [D, N], F16, kind="ExternalInput")
    vT = nc.dram_tensor("vT", [D, N], F16, kind="ExternalInput")
    wqkv_d = nc.dram_tensor("wqkv", [D, 3 * DL], F16, kind="ExternalInput")
    wo_d = nc.dram_tensor("wo", [DL, D], F16, kind="ExternalInput")
    bqk = nc.dram_tensor("bqk", [DL, 3], F32, kind="ExternalInput")
    out = nc.dram_tensor("out", [N, D], F16, kind="ExternalOutput")

    AF = mybir.ActivationFunctionType
    from concourse.masks import make_identity

    with TileContext(nc) as tc, ExitStack() as ctx:
        const = ctx.enter_context(tc.tile_pool(name="const", bufs=1))
        big = ctx.enter_context(tc.tile_pool(name="big", bufs=1))
        stream = ctx.enter_context(tc.tile_pool(name="stream", bufs=1))
        vstage = ctx.enter_context(tc.tile_pool(name="vstage", bufs=2))
        probs_pool = ctx.enter_context(tc.tile_pool(name="probs", bufs=8))
        rc_pool = ctx.enter_context(tc.tile_pool(name="rc", bufs=2))
        outp = ctx.enter_context(tc.tile_pool(name="outp", bufs=16))

        # ---- PE pre-roll: cheap matmuls ahead of the first data-dependent
        # ones (the scheduler's issue pipeline works ahead on these) ----
        dummy = const.tile([16, 64], F16)
        nc.vector.memset(dummy[:], 0.0)
        with tc.tile_pool(name="warm_ps", bufs=1, space="PSUM") as warm_ps:
            wps0 = warm_ps.tile([16, 64], F32)
            for _ in range(85):
                nc.tensor.matmul(wps0[:], dummy[:, 0:16], dummy[:],
                                 start=True, stop=True)

        # ---- all DMAs on the sync queue, in exact consumption order; input
        # tiles are split in half (contraction chunks 0-3 / 4-7) so each
        # projection chain can start on the first half ----
        wqkv = const.tile([128, 8, 3 * DL], F16)
        qs = [[stream.tile([128, 4, 512], F16, name=f"q{t}{i}")
               for i in range(2)] for t in range(4)]
        ks = [[stream.tile([128, 4, 512], F16, name=f"k{t}{i}")
               for i in range(2)] for t in range(4)]
        vs = [[stream.tile([128, 4, 512], F16, name=f"v{t}{i}")
               for i in range(2)] for t in range(4)]
        qTr = qT.rearrange("(j p) n -> p j n", p=128)
        kTr = kT.rearrange("(j p) n -> p j n", p=128)
        vTr = vT.rearrange("(j p) n -> p j n", p=128)
        bias_cols = const.tile([128, 3], F32)
        wo = const.tile([128, D], F16)

        nc.sync.dma_start(wqkv[:],
                          wqkv_d.rearrange("(j p) c -> p j c", p=128))
        nc.sync.dma_start(bias_cols[:], bqk[:])
        # t=0 q/k halves interleaved so both projection chains start early
        nc.sync.dma_start(qs[0][0][:], qTr[:, 0:4, 0:512])
        nc.sync.dma_start(ks[0][0][:], kTr[:, 0:4, 0:512])
        nc.sync.dma_start(qs[0][1][:], qTr[:, 4:8, 0:512])
        nc.sync.dma_start(ks[0][1][:], kTr[:, 4:8, 0:512])
        nc.sync.dma_start(vs[0][0][:], vTr[:, 0:4, 0:512])
        nc.sync.dma_start(vs[0][1][:], vTr[:, 4:8, 0:512])
        nc.sync.dma_start(wo[:], wo_d[:])
        for t in range(1, 4):
            for xs, xr in ((qs, qTr), (ks, kTr), (vs, vTr)):
                for h in range(2):
                    nc.sync.dma_start(
                        xs[t][h][:],
                        xr[:, 4 * h:4 * (h + 1), 512 * t:512 * (t + 1)])

        ident = const.tile([128, 128], F32)
        make_identity(nc, ident[:])
        ones64 = const.tile([1, 64], F16)
        nc.vector.memset(ones64[:], 1.0)

        # persistent SBUF state
        QTs = [big.tile([128, 512], F16, name=f"QT{t}") for t in range(4)]
        KTs = [big.tile([128, 512], F16, name=f"KT{t}") for t in range(4)]
        Vaug0 = big.tile([128, 16, 65], F16)
        Vaug1 = big.tile([128, 16, 65], F16)
        nc.vector.memset(Vaug0[:, :, 64:65], 1.0)
        nc.vector.memset(Vaug1[:, :, 64:65], 1.0)
        attnT = big.tile([128, N], F16)

        def wo_block(m, pool):
            ob = outp.tile([128, 1024], F16, name="ob")
            for u in range(2):
                wps = pool.tile([128, 512], F32, name="mm")
                nc.tensor.matmul(wps[:], attnT[:, 128 * m:128 * (m + 1)],
                                 wo[:, 512 * u:512 * (u + 1)],
                                 start=True, stop=True)
                nc.vector.tensor_copy(ob[:, 512 * u:512 * (u + 1)],
                                      wps[:])
            nc.sync.dma_start(out[128 * m:128 * (m + 1), :], ob[:])

        with tc.tile_pool(name="sc_ps", bufs=2, space="PSUM") as sc_ps, \
             tc.tile_pool(name="pv_ps", bufs=2, space="PSUM") as pv_ps, \
             tc.tile_pool(name="mm_ps", bufs=2, space="PSUM") as mm_ps:

            def qk_proj(t, interleave=False):
                specs = ((0, qs[t], 0, QTs[t]), (DL, ks[t], 1, KTs[t]))
                if not interleave:
                    for c0, src, bcol, dst in specs:
                        ps = mm_ps.tile([128, 512], F32, name="mm")
                        for j in range(8):
                            nc.tensor.matmul(ps[:], wqkv[:, j, c0:c0 + DL],
                                             src[j // 4][:, j % 4, :],
                                             start=(j == 0), stop=(j == 7))
                        nc.vector.tensor_scalar_add(
                            dst[:], ps[:], bias_cols[:, bcol:bcol + 1])
                    return
                # consume input halves in arrival order: Qlo, Klo, Qhi, Khi
                pss = [mm_ps.tile([128, 512], F32, name="mm")
                       for _ in range(2)]
                for half in range(2):
                    for i, (c0, src, bcol, dst) in enumerate(specs):
                        for j in range(4 * half, 4 * half + 4):
                            nc.tensor.matmul(pss[i][:],
                                             wqkv[:, j, c0:c0 + DL],
                                             src[j // 4][:, j % 4, :],
                                             start=(j == 0), stop=(j == 7))
                for i, (c0, src, bcol, dst) in enumerate(specs):
                    nc.vector.tensor_scalar_add(
                        dst[:], pss[i][:], bias_cols[:, bcol:bcol + 1])

            def v_proj(t):
                ps = mm_ps.tile([128, 512], F32, name="mm")
                for j in range(8):
                    nc.tensor.matmul(ps[:], wqkv[:, j, 2 * DL:3 * DL],
                                     vs[t][j // 4][:, j % 4, :],
                                     start=(j == 0), stop=(j == 7))
                vtt = vstage.tile([128, 512], F32, name="vtt")
                nc.vector.tensor_scalar_add(vtt[:], ps[:], bias_cols[:, 2:3])
                for bb in range(4):
                    b = 4 * t + bb
                    tp = mm_ps.tile([128, 512], F32, name="mm")
                    nc.tensor.transpose(tp[:, 0:128],
                                        vtt[:, 128 * bb:128 * (bb + 1)],
                                        ident[:])
                    nc.vector.tensor_copy(Vaug0[:, b, 0:64], tp[:, 0:64])
                    nc.vector.tensor_copy(Vaug1[:, b, 0:64], tp[:, 64:128])

            def score_mm(t, h, kb, sc, off):
                nc.tensor.matmul(
                    sc[:, kb % 2, off:512],
                    KTs[kb // 4][64 * h:64 * (h + 1),
                                 128 * (kb % 4):128 * (kb % 4 + 1)],
                    QTs[t][64 * h:64 * (h + 1), off:512],
                    start=True, stop=True, tile_position=(64 * h, 0))

            def drain_pv(t, h, pvh, pend, keep, max_kb=None):
                # kb-sorted: kb=0 must go first (it owns has_written for the
                # whole accumulation); max_kb guards blocks whose V isn't
                # projected yet
                nkb = 4 * t + 4
                pend.sort(key=lambda e: e[0])
                while len(pend) > keep and (max_kb is None
                                            or pend[0][0] < max_kb):
                    pkb, ppr, pgi, poff = pend.pop(0)
                    nc.tensor.matmul(pvh[:, poff:512],
                                     Vaug_[h][:, pkb, :],
                                     ppr[:, pgi, poff:512],
                                     start=(pkb == 0), stop=(pkb == nkb - 1))

            Vaug_ = (Vaug0, Vaug1)

            # one iteration = one 512-column query tile; emission is ordered
            # by data arrival: full score blocks (need only old V) before the
            # V projection, next Q/K projection before the Wo matmuls
            qk_proj(0, interleave=True)
            for t in range(4):
                nkb = 4 * t + 4
                pvhs = [pv_ps.tile([65, 512], F32, name="pvh")
                        for _ in range(2)]
                pends = [[], []]
                # previous tile's output projections interleave with this
                # tile's full-block scores (their evacs hide behind the PE)
                wo_pend = list(range(4 * (t - 1), 4 * t)) if t >= 1 else []
                # full blocks (kb < 4t), both heads
                for h in range(2):
                    for kb in range(4 * t):
                        if kb % 2 == 0:
                            sc = sc_ps.tile([128, 2, 512], F32, name="sc")
                            pr = probs_pool.tile([128, 2, 512], F16,
                                                 name="pr")
                        score_mm(t, h, kb, sc, 0)
                        if kb % 2 == 1:
                            nc.scalar.activation(pr[:, :, :], sc[:, :, :],
                                                 AF.Exp, scale=0.125)
                            pends[h].append((kb - 1, pr, 0, 0))
                            pends[h].append((kb, pr, 1, 0))
                            if wo_pend and (h > 0 or kb >= 5):
                                wo_block(wo_pend.pop(0), mm_ps)
                        drain_pv(t, h, pvhs[h], pends[h], 4, max_kb=4 * t)
                for m in wo_pend:
                    wo_block(m, mm_ps)

                def diag_scores(t):
                    for h in range(2):
                        for kb in range(4 * t, nkb):
                            if kb % 2 == 0:
                                sc = sc_ps.tile([128, 2, 512], F32,
                                                name="sc")
                                pr = probs_pool.tile([128, 2, 512], F16,
                                                     name="pr")
                            off = 128 * (kb - 4 * t)
                            score_mm(t, h, kb, sc, off)
                            nc.scalar.activation(pr[:, kb % 2, off:512],
                                                 sc[:, kb % 2, off:512],
                                                 AF.Exp, scale=0.125)
                            nc.gpsimd.affine_select(
                                out=pr[:, kb % 2, off:off + 128],
                                in_=pr[:, kb % 2, off:off + 128],
                                compare_op=mybir.AluOpType.is_ge, fill=0.0,
                                base=0, pattern=[[1, 128]],
                                channel_multiplier=-1)
                            pends[h].append((kb, pr, kb % 2, off))

                # t=0: v arrives last, so emit the diagonal scores (which
                # need only Q/K) before the V projection
                if t == 0:
                    diag_scores(t)
                    v_proj(t)
                else:
                    v_proj(t)
                    diag_scores(t)
                # next tile's Q/K projection fills the exp/mask latency
                if t < 3:
                    qk_proj(t + 1)
                # PV drains + softmax finalize, both heads
                for h in range(2):
                    drain_pv(t, h, pvhs[h], pends[h], 0)
                    # softmax normalization: reciprocal of the denominator
                    # row to SBUF, partition-broadcast on gpsimd (SBUF->SBUF
                    # so the multiply reads only one PSUM operand), multiply
                    # into attnT
                    rcr = rc_pool.tile([1, 512], F16, name="rcr")
                    with nc.allow_low_precision(reason="softmax recip row"):
                        nc.vector.reciprocal(rcr[:], pvhs[h][64:65, :])
                    rcb = rc_pool.tile([64, 512], F16, name="rcb")
                    nc.gpsimd.partition_broadcast(rcb[:], rcr[:],
                                                  channels=64)
                    nc.vector.tensor_mul(
                        attnT[64 * h:64 * (h + 1), 512 * t:512 * (t + 1)],
                        pvhs[h][0:64, :], rcb[:])
        # ---- final output projection (row blocks 12..15) in its own PSUM
        # region: the attention pools are closed, so 8 banks decouple the
        # matmuls from their evacuations completely ----
        with tc.tile_pool(name="wo3_ps", bufs=8, space="PSUM") as wo3_ps:
            for m in range(12, 16):
                wo_block(m, wo3_ps)

    nc.compile()
    return nc


def make_in_maps(q, k, v, Wq, bq, Wk, bk, Wv, bv, Wo, bo):
    f16 = np.float16
    f32 = np.float32
    qTa = np.ascontiguousarray(q.T).astype(f16)
    kTa = np.ascontiguousarray(k.T).astype(f16)
    vTa = np.ascontiguousarray(v.T).astype(f16)
    WqT = Wq.T.astype(f16)
    WkT = Wk.T.astype(f16)
    WvT = Wv.T.astype(f16)
    WoT = Wo.T.astype(f16)
    in_maps = []
    for c in range(NCORES):
        d0 = DL * c
        in_maps.append({
            "qT": qTa, "kT": kTa, "vT": vTa,
            "wqkv": np.ascontiguousarray(
                np.concatenate([WqT[:, d0:d0 + DL], WkT[:, d0:d0 + DL],
                                WvT[:, d0:d0 + DL]], axis=1)),
            "wo": np.ascontiguousarray(WoT[d0:d0 + DL, :]),
            "bqk": np.ascontiguousarray(
                np.stack([bq[d0:d0 + DL], bk[d0:d0 + DL], bv[d0:d0 + DL]],
                         axis=1)).astype(f32),
        })
    return in_maps


_NC_CACHE = None


def _get_nc():
    global _NC_CACHE
    if _NC_CACHE is None:
        _NC_CACHE = build_nc()
    return _NC_CACHE


def kernel(q, k, v, Wq, bq, Wk, bk, Wv, bv, Wo, bo):
    """Full-input / full-output entry point (harness contract)."""
    q, k, v = np.asarray(q), np.asarray(k), np.asarray(v)
    Wq, bq, Wk, bk = np.asarray(Wq), np.asarray(bq), np.asarray(Wk), np.asarray(bk)
    Wv, bv, Wo, bo = np.asarray(Wv), np.asarray(bv), np.asarray(Wo), np.asarray(bo)
    nc = _get_nc()
    in_maps = make_in_maps(q, k, v, Wq, bq, Wk, bk, Wv, bv, Wo, bo)
    res = run_bass_kernel_spmd(nc, in_maps, list(range(NCORES)))
    acc = res.results[0]["out"].astype(np.float64)
    for c in range(1, NCORES):
        acc += res.results[c]["out"]
    acc += bo.astype(np.float64)
    return acc.astype(np.float32)
